# revision 43
# baseline (speedup 1.0000x reference)
"""Trainium2 Bass kernel for nn_Attn_58669253263845 (sparse_attention).

Reference computation:
    hidden2 = concat(hidden[0], hidden[1])                 # [B, 2H]
    attn_input = concat(bcast(hidden2), encoder_outputs)   # [B, S, 3H]
    energy = attn_input @ W.T + b                          # [B, S, H]
    scores = energy @ v                                    # [B, S]
    out = softmax(scores, axis=S)

Everything before the softmax is linear, so
    scores[b,s] = attn_input[b,s,:] . (v @ W) + v.b
                = hidden2[b,:] . w_hid + enc[b,s,:] . w_enc + v.b
The hidden/bias terms are constant per batch row and cancel in the softmax
over S.  Hence:
    out = softmax_s(enc[b,s,:] . w_enc),  w_enc = v @ W[:, 2H:3H]

The weight fold (1024x1024 matvec, weights only) is done on host in fp64;
the heavy part (64*512 dot products of length 1024 + softmax) runs on 8
NeuronCores, data-parallel over batch (8 batches per core).

Kernel shape (per core): the kernel is DMA-bound -- it must stream
8 batches x 512 x 1024 encoder values through SBUF once.  Two levers:

 1. fp16 on the wire.  enc and w_enc are rounded to fp16 on host,
    halving HBM->SBUF traffic.  Scores have std ~10 and fp16 rounding
    perturbs them by ~3e-3, an order of magnitude inside the 2e-2
    correctness gate (fp16 x fp16 products accumulate exactly in the
    PE's fp32 accumulators).
 2. dots on the PE, h on partitions.  The host uploads enc pre-permuted
    to [(j,b), p, (hb, s)] chunks (h = 128*hb + p, s_global = 128*j + s),
    so every chunk is one contiguous 256 KiB DMA and the chunk's dot
    products become 8 PSUM-accumulated stationary loads:
        scores[s, (j,b)] += chunk[:, hb]^T_{128x128} @ w[hb]_{128x1}
    The moving side is a single w column, so the PE trails the DMA
    stream with almost no engine time, and the DVE (whose fused dot
    gets no 16-bit speedup) drops out of the streaming path entirely.

Chunks stream j-major (all 8 batches of s-group j consecutively) as
512 KiB chunk-pairs, except the last s-group which arrives as 256 KiB
singles so the tail only waits on batch 7's chunk.  The per-group
epilogue (PSUM->SBUF score copy, PE transpose to batch-major, exp+accum
on ACT) overlaps the remaining stream for j < 3; only s-group 3's
epilogue plus the final reciprocal+scale sit behind the last chunk.
Each s-group's scores accumulate in their own PSUM bank (PSUM
dependency tracking is bank-granular; a shared bank would chain every
epilogue behind the final group's matmuls).
"""

import sys
import types

import numpy as np
import concourse.bacc as bacc
import concourse.bass as bass
import concourse.mybir as mybir
import concourse.tile as tile
from concourse.bass_utils import run_bass_kernel_spmd

# run_bass_kernel_spmd(trace=True) (e.g. via BASS_TRACE=1 in the env)
# imports antenv.axon_hooks, which does not exist in this container. Register
# a stub returning "no hook" so tracing degrades gracefully instead of
# raising ModuleNotFoundError.
try:
    import antenv.axon_hooks  # noqa: F401
except ImportError:
    try:
        import antenv

        _stub = types.ModuleType("antenv.axon_hooks")
        _stub.get_axon_ntff_profile_hook = lambda: None  # type: ignore[attr-defined]
        sys.modules["antenv.axon_hooks"] = _stub
        antenv.axon_hooks = _stub
    except ImportError:
        pass

N_CORES = 8
B, S, H = 64, 512, 1024
P = 128             # SBUF partitions
BPC = B // N_CORES  # batches per core = 8
HB = H // P         # h-blocks per dot = 8
JT = S // P         # s-groups per batch = 4

F32 = mybir.dt.float32
F16 = mybir.dt.float16

_compiled_nc = None
LAST_RESULTS = None  # BassKernelResults of the most recent run (for profiling)




def _build_nc():
    """Per-core kernel: probs[BPC, S] = softmax_s(enc[BPC, S, H] @ w_enc).

    enc arrives pre-permuted as [(j,b), P, HB*P] fp16 (chunk (j,b) holds
    s-group j of batch b, h-within-block on partitions, (hb, s) on free),
    w_enc as [P, HB] fp16 (column hb = h-block hb's 128 weights).
    """
    # Bacc (not raw Bass): its compile() legalizes multi-wait instructions
    # into EventSemaphore waits (TRN2 allows only 1 sync wait per inst).
    nc = bacc.Bacc("TRN2", target_bir_lowering=False, debug=False)

    NCHUNK = JT * BPC
    NPAIR = (JT - 1) * BPC // 2
    # Groups 0..JT-2 arrive as 512 KiB chunk-pairs (fewer DMAs, more issue-
    # pipeline margin); the last group arrives as 256 KiB singles so the tail
    # only waits on batch 7's chunk. The host packs each pair partition-
    # interleaved ([p, (which, hb, s)]) because a DMA maps DRAM to the SBUF
    # tile by FLAT element order.
    pairs_d = nc.dram_tensor(
        "enc_pairs", [NPAIR, P, 2 * HB * P], F16, kind="ExternalInput"
    )
    tail_d = nc.dram_tensor("enc_tail", [BPC, P, HB * P], F16, kind="ExternalInput")
    w_d = nc.dram_tensor("w_in", [P, HB], F16, kind="ExternalInput")
    out_d = nc.dram_tensor("probs_out", [BPC, S], F32, kind="ExternalOutput")

    encp = pairs_d.ap()
    enct = tail_d.ap()

    with tile.TileContext(nc) as tc:
        with (
            tc.tile_pool(name="const", bufs=1) as constp,
            tc.tile_pool(name="ebuf", bufs=NCHUNK) as ebufp,
            tc.tile_pool(name="small", bufs=1) as smallp,
            tc.tile_pool(name="psum", bufs=1, space="PSUM") as psump,
        ):
            w_sb = constp.tile([P, HB], F16, name="w_sb")

            # identity for the PE transposes, built on-device (gpsimd is idle
            # and this keeps 64KiB off the serial DMA stream).
            ones_id = constp.tile([P, P], F32, name="ones_id")
            nc.gpsimd.memset(ones_id[:], 1.0)
            id_t = constp.tile([P, P], F32, name="id_t")
            nc.gpsimd.affine_select(
                out=id_t[:],
                in_=ones_id[:],
                pattern=[[-1, P]],
                compare_op=mybir.AluOpType.is_equal,
                fill=0.0,
                channel_multiplier=1,
            )

            # scores_j[s, b] accumulate over the 8 h-blocks of each chunk.
            # One PSUM tile PER s-group: PSUM dependency tracking is
            # bank-granular, so a shared tile would chain every group's
            # epilogue behind the final group's matmuls.
            scores = [
                psump.tile([P, BPC], F32, name=f"scores{j}", tag=f"scores{j}")
                for j in range(JT)
            ]

            # DMA stream: one 256 KiB chunk per transfer, j-major (all 8
            # batches of an s-group consecutively) so each group's epilogue
            # overlaps the remaining stream.
            # NOTE: one chunk per DMA -- a [2, P, F] -> [P, 2F] transfer maps
            # by FLAT element order (chunk 0 would land on partitions 0-63).
            # w's 2 KiB DMA rides 9th: each 728 ns enc transfer banks 78 ns
            # of issue-pipeline margin (728 transfer vs 650 issue), and w's
            # 650 ns issue slot needs ~8 chunks of margin to hide; the
            # matmuls are 2 ns each and instantly catch up once w lands.
            tiles = {}
            for k in range(NPAIR):
                t = ebufp.tile([P, 2 * HB * P], F16, name=f"ep{k}", tag="e")
                nc.sync.dma_start(t[:], encp[k])
                ci = 2 * k
                tiles[(ci // BPC, ci % BPC)] = t[:, 0 : HB * P]
                tiles[(ci // BPC, ci % BPC + 1)] = t[:, HB * P : 2 * HB * P]
                # w's 2 KiB DMA rides second: a 1456 ns pair transfer banks
                # ~800 ns of issue-pipeline margin over the 650 ns issue
                # cost, which hides w's own 650 ns issue slot.
                if k == 0:
                    nc.sync.dma_start(w_sb[:], w_d.ap())
            for b in range(BPC):
                t = ebufp.tile([P, HB * P], F16, name=f"et{b}", tag="e")
                nc.sync.dma_start(t[:], enct[b])
                tiles[(JT - 1, b)] = t[:]

            # Dot products: 8 accumulated stationary loads per chunk. The
            # moving operand is one w column, so per-matmul engine time is a
            # single column pass.
            for j in range(JT):
                for b in range(BPC):
                    ch = tiles[(j, b)]
                    for hb in range(HB):
                        nc.tensor.matmul(
                            scores[j][:, b : b + 1],
                            ch[:, hb * P : (hb + 1) * P],
                            w_sb[:, hb : hb + 1],
                            start=(hb == 0),
                            stop=(hb == HB - 1),
                        )

            # Per-group epilogue: PSUM -> SBUF copy (PE transpose reads SBUF
            # only), transpose to batch-major, exp+partial row sum. Groups
            # j < 3 overlap the remaining DMA/matmul stream.
            # Each transpose lands in its OWN PSUM bank (PSUM deps are
            # bank-granular; sharing one would serialize the exps).
            scs = smallp.tile([P, NCHUNK], F32, name="scs")
            psumT = [
                psump.tile([BPC, P], F32, name=f"psumT{j}", tag=f"psumT{j}")
                for j in range(JT)
            ]
            expt = smallp.tile([BPC, S], F32, name="expt")
            sums4 = smallp.tile([BPC, JT], F32, name="sums4")
            for j in range(JT):
                cols = slice(j * BPC, (j + 1) * BPC)
                nc.vector.tensor_copy(scs[:, cols], scores[j][:])
                nc.tensor.transpose(psumT[j][:], scs[:, cols], id_t[:])
                # softmax without max-subtraction: |score| < ~60 is far
                # inside fp32 exp range and softmax is shift-invariant.
                nc.scalar.activation(
                    out=expt[:, j * P : (j + 1) * P],
                    in_=psumT[j][:],
                    func=mybir.ActivationFunctionType.Exp,
                    bias=0.0,
                    scale=1.0,
                    accum_out=sums4[:, j : j + 1],
                )

            sums = smallp.tile([BPC, 1], F32, name="sums")
            nc.vector.tensor_reduce(
                out=sums[:],
                in_=sums4[:],
                axis=mybir.AxisListType.X,
                op=mybir.AluOpType.add,
            )
            binv = smallp.tile([BPC, 1], F32, name="binv")
            nc.vector.reciprocal(binv[:], sums[:])
            prob = smallp.tile([BPC, S], F32, name="prob")
            nc.vector.tensor_scalar_mul(prob[:], expt[:], binv[:])

            nc.sync.dma_start(out_d.ap(), prob[:])

    nc.finalize()  # Bacc: runs compile() (wait legalization, reg alloc, ...)
    return nc


def kernel(hidden, encoder_outputs, W, b, v):
    global _compiled_nc, LAST_RESULTS

    # Fold the linear layer on host (fp64 for accuracy): only the
    # encoder-input slice of W survives the softmax. Force numpy so the fold
    # never runs through a jax device backend.
    W = np.asarray(W)
    v = np.asarray(v)
    w_enc = (v.astype(np.float64) @ W[:, 2 * H :].astype(np.float64)).astype(
        np.float32
    )
    # [P, HB] fp16: column hb holds weights for h = 128*hb .. 128*hb+127.
    w_t = np.ascontiguousarray(w_enc.astype(np.float16).reshape(HB, P).T)
    # Per-core chunk layout [(j,b), p, (hb, s)]: each (s-group, batch) chunk
    # is contiguous with h-within-block on partitions, so chunk DMAs are
    # plain contiguous transfers and the PE contracts over the partition dim.
    enc = np.asarray(encoder_outputs, dtype=np.float32).astype(np.float16)
    # [B, S, H] -> [B, JT, 128s, HB, 128p] -> [B, JT, 128p, HB, 128s]
    enc = enc.reshape(B, JT, P, HB, P).transpose(0, 1, 4, 3, 2)

    if _compiled_nc is None:
        _compiled_nc = _build_nc()

    NPAIR = (JT - 1) * BPC // 2
    in_maps = []
    for c in range(N_CORES):
        # [BPC, JT, p, hb, s] -> [(j, b), p, (hb, s)]
        core = enc[c * BPC : (c + 1) * BPC].transpose(1, 0, 2, 3, 4)
        core = core.reshape(JT * BPC, P, HB * P)
        # Groups 0..JT-2 as partition-interleaved chunk-pairs (a DMA maps
        # DRAM to SBUF by flat order): [pair, p, (which, hb, s)].
        pairs = (
            core[: 2 * NPAIR]
            .reshape(NPAIR, 2, P, HB * P)
            .transpose(0, 2, 1, 3)
            .reshape(NPAIR, P, 2 * HB * P)
        )
        in_maps.append(
            {
                "enc_pairs": np.ascontiguousarray(pairs),
                "enc_tail": np.ascontiguousarray(core[2 * NPAIR :]),
                "w_in": w_t,
            }
        )
    LAST_RESULTS = run_bass_kernel_spmd(
        _compiled_nc, in_maps, core_ids=list(range(N_CORES))
    )
    out = np.concatenate([r["probs_out"] for r in LAST_RESULTS.results], axis=0)
    return out.astype(np.float32)
